# revision 34
# baseline (speedup 1.0000x reference)
"""Causal self-attention (GQA + RoPE) for TRN2, sharded over 8 NeuronCores.

Sharding: tensor-parallel over heads. Each core owns 4 query heads and 1 KV
head (H=32, HKV=8 -> group size 4). Column-parallel q/k/v projections,
row-parallel o_proj; the final all-reduce over the 8 partial [T, D] outputs
happens on the host after the gather.

v2 (bf16 pipeline):
  - All matmul operands are bf16 (PSUM accumulation stays fp32). bf16 enables
    Fast Weight Load on LDWEIGHTS (fp32/f32r is excluded from FWL), halves
    SBUF/DMA traffic, and doubles DVE throughput on SBUF-only elementwise ops.
  - Attention runs per head-PAIR (heads 2hp, 2hp+1 live on partition halves
    0:64 / 64:128 of qT[hp]): the two score matmuls of a pair use disjoint
    PE row-groups (tile_position rows 0 vs 64) and overlap on the array.
  - exp runs once per (pair, key-block) over a [128, 2, 512] PSUM group,
    amortizing the ACT engine's 352-cycle pipeline-fill across both heads.
  - Scores are computed transposed (scoresT [s, t]) so the softmax denominator
    rides the attn@v matmul via a ones-column appended to the v stationary.
  - Causal masking: above-diagonal s-blocks are skipped; diagonal blocks get
    one affine_select over both heads post-exp.
  - 1/denom (from ACT's LUT reciprocal) is broadcast across partitions with
    gpsimd.partition_broadcast, keeping the PE and PSUM out of the epilogue.
  - o_proj evictions run on DVE (ACT is loaded with exp); out is written bf16
    in a tiled [ti, n, p, s] DRAM layout (contiguous 128KB stores) and
    assembled + reduced on the host.
  - x is staged in DRAM pre-tiled per (k-chunk, strip) so every activation
    load is one contiguous 128KB DMA.
"""

import math

import numpy as np

import concourse.bass as bass
import concourse.mybir as mybir
import concourse.tile as tile
from concourse import bacc
from concourse.masks import make_identity

D = 2048
H = 32
HKV = 8
HD = 64
T = 2048
NCORES = 8
HPC = H // NCORES        # 4 query heads per core
QC = HPC * HD            # 256 q dims per core
ROPE_BASE = 10000.0
S = 512                  # t-strip / moving-operand width
NSTRIP = T // S          # 4
KC = D // 128            # 16 contraction chunks

F32 = mybir.dt.float32
F32R = mybir.dt.float32r
BF16 = mybir.dt.bfloat16


def _build_kernel(debug=False):
    nc = bacc.Bacc("TRN2", target_bir_lowering=False, debug=False,
                   num_devices=NCORES)

    # x pre-tiled: rows (kc*NSTRIP + strip)*128 + p, cols s
    xTt = nc.dram_tensor("xTt", [KC * NSTRIP * 128, S], BF16,
                         kind="ExternalInput").ap()
    wqT = nc.dram_tensor("wqT", [D, QC], BF16, kind="ExternalInput").ap()
    wkvT = nc.dram_tensor("wkvT", [D, 128], BF16, kind="ExternalInput").ap()
    woT3 = nc.dram_tensor("woT3", [128, 2, D], BF16, kind="ExternalInput").ap()
    cosT = nc.dram_tensor("cosT", [128, T], BF16, kind="ExternalInput").ap()
    sinT = nc.dram_tensor("sinT", [128, T], BF16, kind="ExternalInput").ap()
    # out pre-tiled: rows ((ti*4 + n)*128 + p), cols s  (ti = global 128-row
    # tile of t, n = 512-col block of d)
    out = nc.dram_tensor("out", [T // 128 * 4 * 128, S], BF16,
                         kind="ExternalOutput").ap()
    dbg = {}
    if debug:
        for nm, shp, dt in [("d_qT0", [128, T], BF16), ("d_qT1", [128, T], BF16),
                            ("d_kT", [128, T], BF16),
                            ("d_vaug", [128, 16 * 65], BF16),
                            ("d_ex", [128, 2 * 4 * S], BF16),
                            ("d_yt", [128, 2 * S], F32),
                            ("d_dn", [128, S], F32), ("d_bc", [128, S], F32),
                            ("d_ytn0", [128, S], BF16),
                            ("d_ytn1", [128, S], BF16)]:
            dbg[nm] = nc.dram_tensor(nm, shp, dt, kind="ExternalOutput").ap()

    def act_reciprocal(out_ap, in_ap):
        """ACT-engine reciprocal via direct InstActivation emission.
        bass gatekeeps ActivationFunctionType.Reciprocal behind a precision
        advisory; for softmax denominators the LUT accuracy (~1e-5) is far
        below the bf16 noise floor, and ACT has idle slots between exps."""
        eng = nc.scalar
        ins = [eng.lower_ap(in_ap)]
        for v in (0.0, 1.0, 0.0):  # bias, scale, alpha
            ins.append(mybir.ImmediateValue(dtype=mybir.dt.float32, value=v))
        return eng.add_instruction(mybir.InstActivation(
            name=nc.get_next_instruction_name(),
            func=mybir.ActivationFunctionType.Reciprocal,
            ins=ins, outs=[eng.lower_ap(out_ap)]))

    with tile.TileContext(nc) as tc:
        with (
            tc.tile_pool(name="consts", bufs=1) as consts,
            tc.tile_pool(name="persist", bufs=1) as persist,
            tc.tile_pool(name="xa", bufs=50) as xap,
            tc.tile_pool(name="rtmp", bufs=6) as rtmp,
            tc.tile_pool(name="swp", bufs=4) as swp,
            tc.tile_pool(name="vtmp", bufs=2) as vtmp,
            tc.tile_pool(name="expp", bufs=4) as expp,
            tc.tile_pool(name="ytn", bufs=6) as ytnp,
            tc.tile_pool(name="outst", bufs=4) as outst,
            tc.tile_pool(name="dn", bufs=4) as dnp,
            tc.tile_pool(name="psA", bufs=1, space="PSUM") as psA,   # 4KB
            tc.tile_pool(name="psY", bufs=2, space="PSUM") as psY,   # 8KB
            tc.tile_pool(name="psP", bufs=1, space="PSUM") as psP,   # 2KB
            tc.tile_pool(name="psO", bufs=1, space="PSUM") as psO,   # 2KB
        ):
            # ---- constants; DMA issue order interleaved per k-chunk so the
            # first projection matmul only waits on chunk 0 of wq/x ----
            wq_sb = consts.tile([128, KC, QC], BF16)
            wkv_sb = consts.tile([128, KC, 128], BF16)
            xa_strips = {}

            def load_xa(strip):
                xa = []
                for kc in range(KC):
                    xt = xap.tile([128, S], BF16, tag="xa",
                                  name=f"xa{strip}_{kc}")
                    r0 = (kc * NSTRIP + strip) * 128
                    nc.sync.dma_start(out=xt, in_=xTt[r0:r0 + 128, :])
                    xa.append(xt)
                xa_strips[strip] = xa

            xa0 = []
            for kc in range(KC):
                nc.sync.dma_start(
                    out=wq_sb[:, kc, :], in_=wqT[kc * 128:(kc + 1) * 128, :])
                nc.sync.dma_start(
                    out=wkv_sb[:, kc, :], in_=wkvT[kc * 128:(kc + 1) * 128, :])
                xt = xap.tile([128, S], BF16, tag="xa", name=f"xa0_{kc}")
                r0 = (kc * NSTRIP + 0) * 128
                nc.sync.dma_start(out=xt, in_=xTt[r0:r0 + 128, :])
                xa0.append(xt)
            xa_strips[0] = xa0
            cs_c = consts.tile([128, T], BF16)
            cs_s = consts.tile([128, T], BF16)
            nc.sync.dma_start(out=cs_c, in_=cosT)
            nc.sync.dma_start(out=cs_s, in_=sinT)
            load_xa(1)
            wo_sb = consts.tile([128, 2, D], BF16)
            nc.gpsimd.dma_start(out=wo_sb, in_=woT3)
            ident = consts.tile([128, 128], F32)
            make_identity(nc, ident)
            # PE warmup: ~3.4us of array activity lifts the HAM 1.2GHz cold
            # throttle while the first input DMAs land (fp32 on purpose:
            # 4 cyc/row keeps the array busy longer per instruction)
            warm_ps = psO.tile([128, S], F32, tag="po", name="warm")
            junk = consts.tile([128, S], F32)
            nc.vector.memset(junk, 1.0)
            for w in range(7):
                nc.tensor.matmul(
                    warm_ps, ident, junk,
                    start=True, stop=True, skip_group_check=True)

            # persistent activations
            qT = [persist.tile([128, T], BF16, tag=f"qT{i}", name=f"qT{i}")
                  for i in range(2)]
            # k duplicated on both partition halves so each q head can use
            # a stationary slice whose base partition matches its rhs base
            kT = persist.tile([128, T], BF16)
            vaug = persist.tile([128, 4 * NSTRIP, 65], BF16)
            ones_col = consts.tile([128, 4 * NSTRIP, 1], BF16)
            nc.vector.memset(ones_col, 1.0)
            nc.vector.tensor_copy(vaug[:, :, 64:65], ones_col)
            ones_f = consts.tile([128, 64], F32)
            nc.vector.memset(ones_f, 1.0)
            ones_r = consts.tile([128, 64], F32R)
            nc.vector.tensor_copy(ones_r, ones_f)

            def proj_filler(strip):
                """Yield closures, each emitting one PE op of this strip's
                q/kv projection; rope/evict DVE work rides along after the
                last matmul of each accumulation group."""
                t0 = strip * S
                tsl = slice(t0, t0 + S)
                xa = xa_strips[strip]

                def rope_q(hp, pq):
                    qc = rtmp.tile([128, S], BF16, tag="rtmp",
                                   name=f"qc{strip}{hp}")
                    qs = rtmp.tile([128, S], BF16, tag="rtmp",
                                   name=f"qs{strip}{hp}")
                    nc.vector.tensor_mul(qc, pq, cs_c[:, tsl])
                    nc.vector.tensor_mul(qs, pq, cs_s[:, tsl])
                    sw = swp.tile([128, S], BF16, tag="swp",
                                  name=f"sw{strip}{hp}")
                    for b in range(2):
                        nc.gpsimd.dma_start(
                            out=sw[b * 64:b * 64 + 32, :],
                            in_=qs[b * 64 + 32:b * 64 + 64, :])
                        nc.gpsimd.dma_start(
                            out=sw[b * 64 + 32:b * 64 + 64, :],
                            in_=qs[b * 64:b * 64 + 32, :])
                    nc.vector.tensor_add(qT[hp][:, tsl], qc, sw)

                for hp in range(2):
                    pool = psP if hp == 0 else psO
                    tag = "proj" if hp == 0 else "po"
                    pq = pool.tile([128, S], F32, tag=tag,
                                   name=f"pq{strip}_{hp}")
                    for kc in range(KC):
                        def mk(hp=hp, pq=pq, kc=kc):
                            nc.tensor.matmul(
                                pq, wq_sb[:, kc, hp * 128:(hp + 1) * 128],
                                xa[kc], start=(kc == 0), stop=(kc == KC - 1))
                            if kc == KC - 1:
                                rope_q(hp, pq)
                        yield mk

                pkv = psP.tile([128, S], F32, tag="proj", name=f"pkv{strip}")

                def rope_kv():
                    kc_t = rtmp.tile([128, S], BF16, tag="rtmp",
                                     name=f"kc{strip}")
                    ks_t = rtmp.tile([128, S], BF16, tag="rtmp",
                                     name=f"ks{strip}")
                    nc.vector.tensor_mul(
                        kc_t[0:64, :], pkv[0:64, :], cs_c[0:64, tsl])
                    nc.vector.tensor_mul(
                        ks_t[0:64, :], pkv[0:64, :], cs_s[0:64, tsl])
                    swk = swp.tile([128, S], BF16, tag="swp",
                                   name=f"swk{strip}")
                    nc.gpsimd.dma_start(out=swk[0:32, :], in_=ks_t[32:64, :])
                    nc.gpsimd.dma_start(out=swk[32:64, :], in_=ks_t[0:32, :])
                    nc.vector.tensor_add(
                        kT[0:64, tsl], kc_t[0:64, :], swk[0:64, :])
                    nc.gpsimd.dma_start(out=kT[64:128, tsl], in_=kT[0:64, tsl])
                    vt_s = vtmp.tile([128, S], F32, tag="vtmp",
                                     name=f"vt{strip}")
                    nc.vector.tensor_copy(vt_s[64:128, :], pkv[64:128, :])
                    return vt_s

                state = {}
                for kc in range(KC):
                    def mk(kc=kc):
                        nc.tensor.matmul(
                            pkv, wkv_sb[:, kc, :], xa_strips[strip][kc],
                            start=(kc == 0), stop=(kc == KC - 1))
                        if kc == KC - 1:
                            state["vt_s"] = rope_kv()
                    yield mk
                for n in range(4):
                    def mk(n=n):
                        pt = psO.tile([128, S], F32, tag="po",
                                      name=f"pt{strip}{n}")
                        nc.tensor.transpose(
                            pt[:, 0:64],
                            state["vt_s"][64:128, n * 128:(n + 1) * 128],
                            ident[64:128, 64:128])
                        nc.vector.tensor_copy(
                            vaug[:, strip * 4 + n, 0:64], pt[:, 0:64])
                    yield mk

            def oproj_filler(strip, ytn, alt=False):
                """Yield closures, each emitting one o_proj matmul; the
                eviction + store ride along after each group's stop. With
                alt=True (legal only while no projection holds psP) po tiles
                alternate between the two 1-buf pools so consecutive tiles
                don't serialize on each other's eviction."""
                for tsub in range(4):
                    ti = strip * 4 + tsub
                    for n in range(4):
                        if alt and n % 2 == 1:
                            po = psP.tile([128, S], F32, tag="proj",
                                          name=f"po{strip}{tsub}{n}")
                        else:
                            po = psO.tile([128, S], F32, tag="po",
                                          name=f"po{strip}{tsub}{n}")
                        for c in range(2):
                            def mk(po=po, c=c, tsub=tsub, n=n, ti=ti):
                                nc.tensor.matmul(
                                    po,
                                    ytn[c][:, tsub * 128:(tsub + 1) * 128],
                                    wo_sb[:, c, n * S:(n + 1) * S],
                                    start=(c == 0), stop=(c == 1),
                                    skip_group_check=True)
                                if c == 1:
                                    ot = outst.tile(
                                        [128, S], BF16, tag="out",
                                        name=f"ot{strip}{tsub}{n}")
                                    nc.vector.tensor_copy(ot, po)
                                    r0 = (ti * 4 + n) * 128
                                    nc.sync.dma_start(
                                        out=out[r0:r0 + 128, :], in_=ot)
                            yield mk

            def run_filler(filler, frac):
                import itertools
                for fn in itertools.islice(filler, frac):
                    fn()

            # strip 0 projection runs dense (nothing to overlap with)
            for fn in proj_filler(0):
                fn()

            ytn_strips = {}
            fillers = []

            for strip in range(NSTRIP):
                t0 = strip * S
                n_sc = (strip + 1) * 4
                ytn = [ytnp.tile([128, S], BF16, tag="ytn",
                                 name=f"ytn{strip}{i}") for i in range(2)]
                ytn_strips[strip] = ytn

                # filler schedule, balanced so the long strip-3 chain still
                # has PE work to hide exp latency: strip0 <- proj(1);
                # strip1 <- proj(2)+oproj(0); strip2 <- proj(3);
                # strip3 <- oproj(1)+oproj(2); tail <- oproj(3)
                pending = 0
                if strip + 1 < NSTRIP:
                    if strip + 2 < NSTRIP:
                        load_xa(strip + 2)
                    fillers.append(proj_filler(strip + 1))
                    pending += 52
                for op_strip in ([0] if strip == 2 else
                                 [1, 2] if strip == 3 else []):
                    fillers.append(oproj_filler(op_strip,
                                                ytn_strips[op_strip],
                                                alt=(strip == 3)))
                    pending += 32

                n_chunks = 2 * n_sc

                import itertools
                filler_iter = itertools.chain(*fillers)
                fillers = [filler_iter]
                emitted = [0]
                chunk_i = [0]

                def chunk_filler():
                    # front-loaded drain: fillers done by ~85% of chunks so
                    # rope/o_proj chains complete well before the strip ends
                    chunk_i[0] += 1
                    tgt = min(n_chunks, chunk_i[0] * 5 // 4)
                    want = pending * tgt // n_chunks - emitted[0]
                    emitted[0] += want
                    run_filler(filler_iter, want)

                ytps = []
                for hp in range(2):          # head pair (2hp, 2hp+1)
                    ytp = psY.tile([128, 2, S], F32, tag="yt",
                                   name=f"ytp{strip}{hp}")
                    ytps.append(ytp)
                    for j in range(n_sc):
                        o = max(j * 128 - t0, 0)
                        jb = slice(j * 128, (j + 1) * 128)
                        # in the projection-free last strip, odd key-blocks
                        # borrow the two idle 1-bank pools so consecutive
                        # blocks' scores don't serialize on a single psA
                        # buffer behind exp
                        split = (strip == NSTRIP - 1) and (j % 2 == 1)
                        if split:
                            pss = [psO.tile([128, S], F32, tag="po",
                                            name=f"sA{strip}{hp}{j}"),
                                   psP.tile([128, S], F32, tag="proj",
                                            name=f"sB{strip}{hp}{j}")]
                            ps_h = [pss[0][:, o:S], pss[1][:, o:S]]
                        else:
                            ps = psA.tile([128, 2, S], F32, tag="ps",
                                          name=f"s{strip}{hp}{j}")
                            ps_h = [ps[:, 0, o:S], ps[:, 1, o:S]]
                        # the two heads use disjoint PE row groups (0 / 64)
                        # and overlap on the array
                        nc.tensor.matmul(
                            ps_h[0], kT[0:64, jb],
                            qT[hp][0:64, t0 + o:t0 + S],
                            start=True, stop=True)
                        nc.tensor.matmul(
                            ps_h[1], kT[64:128, jb],
                            qT[hp][64:128, t0 + o:t0 + S],
                            start=True, stop=True)
                        ex = expp.tile([128, 2, S], BF16, tag="exp",
                                       name=f"e{strip}{hp}{j}")
                        if split:
                            for h01 in range(2):
                                nc.scalar.activation(
                                    ex[:, h01, o:S], ps_h[h01],
                                    mybir.ActivationFunctionType.Exp,
                                    scale=1.0 / math.sqrt(HD))
                        else:
                            nc.scalar.activation(
                                ex[:, :, o:S], ps[:, :, o:S],
                                mybir.ActivationFunctionType.Exp,
                                scale=1.0 / math.sqrt(HD))
                        if j * 128 - t0 >= 0:
                            nc.gpsimd.affine_select(
                                out=ex[:, :, o:o + 128],
                                in_=ex[:, :, o:o + 128],
                                pattern=[[0, 2], [1, 128]], base=0,
                                channel_multiplier=-1,
                                compare_op=mybir.AluOpType.is_ge, fill=0.0)
                        if debug and strip == 0 and hp == 0:
                            nc.sync.dma_start(
                                out=dbg["d_ex"].rearrange(
                                    "p (j h s) -> p j h s", j=4, h=2)[:, j],
                                in_=ex)
                        chunk_filler()
                        nc.tensor.matmul(
                            ytp[0:65, 0, o:S], vaug[:, j, :], ex[:, 0, o:S],
                            start=(j == 0), stop=(j == n_sc - 1),
                            skip_group_check=True)
                        nc.tensor.matmul(
                            ytp[0:65, 1, o:S], vaug[:, j, :], ex[:, 1, o:S],
                            start=(j == 0), stop=(j == n_sc - 1),
                            skip_group_check=True)

                if debug and strip == 0:
                    ydump = dnp.tile([128, 2, S], F32, tag="ydump",
                                     bufs=1, name="ydump")
                    nc.vector.tensor_copy(ydump, ytps[0])
                    nc.sync.dma_start(
                        out=dbg["d_yt"],
                        in_=ydump.rearrange("p a b -> p (a b)"))
                # strip epilogue: 1/denom for all 4 heads via ACT LUT recip
                # (two adjacent instructions -> one Exp->Recip->Exp table
                # round-trip per strip), broadcast across partitions with PE
                # outer products (ones[64] x recip-row) into the scores pool
                # banks, then normalize on DVE
                dds = []
                for hp in range(2):
                    dd = dnp.tile([128, 2, S], F32R, tag="dd", bufs=2,
                                  name=f"dd{strip}{hp}")
                    nc.scalar.copy(dd[64:65, :, :], ytps[hp][64:65, :, :])
                    dds.append(dd)
                run_filler(filler_iter, 5)
                for hp in range(2):
                    bc_ps = psA.tile([128, 2, S], F32, tag="ps",
                                     name=f"bcp{strip}{hp}")
                    for h01 in range(2):
                        nc.tensor.matmul(
                            bc_ps[0:64, h01, :], ones_r[64:65, :],
                            dds[hp][64:65, h01, :],
                            start=True, stop=True, skip_group_check=True)
                    bc_t = dnp.tile([128, 2, S], F32, tag="dn", bufs=2,
                                    name=f"bc{strip}{hp}")
                    nc.scalar.copy(bc_t[0:64, :, :], bc_ps[0:64, :, :])
                    # the custom-DVE approx recip needs a base-0 AP (it
                    # mis-executes on partition-offset slices), so invert the
                    # broadcast denominator on 64 lanes
                    bc_r = dnp.tile([128, 2, S], F32, tag="dr", bufs=2,
                                    name=f"br{strip}{hp}")
                    nc.vector.reciprocal_approx_fast(
                        out=bc_r[0:64, :, :], in_=bc_t[0:64, :, :])
                    nc.vector.tensor_mul(
                        ytn[hp][0:64, :], ytps[hp][0:64, 0, :],
                        bc_r[0:64, 0, :])
                    ntmp = dnp.tile([128, S], BF16, tag="ntmp",
                                    bufs=2, name=f"nt{strip}{hp}")
                    nc.vector.tensor_mul(
                        ntmp[0:64, :], ytps[hp][0:64, 1, :], bc_r[0:64, 1, :])
                    nc.gpsimd.dma_start(
                        out=ytn[hp][64:128, :], in_=ntmp[0:64, :])
                    if debug and strip == 0 and hp == 0:
                        nc.sync.dma_start(out=dbg["d_dn"],
                                          in_=bc_r[:, 0, :])
                        nc.sync.dma_start(out=dbg["d_bc"], in_=bc_t[:, 0, :])

                if debug and strip == 0:
                    nc.sync.dma_start(out=dbg["d_qT0"], in_=qT[0])
                    nc.sync.dma_start(out=dbg["d_qT1"], in_=qT[1])
                    nc.sync.dma_start(out=dbg["d_kT"], in_=kT)
                    nc.sync.dma_start(
                        out=dbg["d_vaug"],
                        in_=vaug.rearrange("p a b -> p (a b)"))
                    nc.sync.dma_start(out=dbg["d_ytn0"], in_=ytn[0])
                    nc.sync.dma_start(out=dbg["d_ytn1"], in_=ytn[1])

                # drain any leftover filler before the next strip
                for fn in filler_iter:
                    fn()
                fillers = []

            # last strip's o_proj runs dense at the tail
            for fn in oproj_filler(NSTRIP - 1, ytn_strips[NSTRIP - 1],
                                   alt=True):
                fn()

    nc.compile()
    return nc


_NC_CACHE = None


def _get_nc():
    global _NC_CACHE
    if _NC_CACHE is None:
        _NC_CACHE = _build_kernel()
    return _NC_CACHE


def _to_bf16(a):
    import ml_dtypes
    return np.ascontiguousarray(a).astype(ml_dtypes.bfloat16)


def _prep_inputs(x, wq, wk, wv, wo):
    """Host-side shard + layout prep. Returns per-core input maps."""
    x = np.asarray(x, dtype=np.float32).reshape(T, D)
    wq = np.asarray(wq, dtype=np.float32)
    wk = np.asarray(wk, dtype=np.float32)
    wv = np.asarray(wv, dtype=np.float32)
    wo = np.asarray(wo, dtype=np.float32)

    xT = np.ascontiguousarray(x.T)  # [D, T]
    xTt = _to_bf16(
        xT.reshape(KC, 128, NSTRIP, S).transpose(0, 2, 1, 3)
        .reshape(KC * NSTRIP * 128, S))

    # head-dim permutation for rope: [even pair comps | odd pair comps]
    perm = np.concatenate([np.arange(0, HD, 2), np.arange(1, HD, 2)])

    # rope tables in the [d, t] layout
    theta = 1.0 / ROPE_BASE ** (np.arange(0, HD, 2, dtype=np.float64) / HD)
    ang = np.arange(T, dtype=np.float64)[None, :] * theta[:, None]  # [32, T]
    cos_blk = np.cos(ang).astype(np.float32)
    sin_blk = np.sin(ang).astype(np.float32)
    cosT = _to_bf16(np.tile(np.concatenate([cos_blk, cos_blk], 0), (2, 1)))
    sinT = _to_bf16(np.tile(np.concatenate([sin_blk, -sin_blk], 0), (2, 1)))

    in_maps = []
    for c in range(NCORES):
        wq_c = wq[c * QC:(c + 1) * QC].reshape(HPC, HD, D)[:, perm, :]
        wq_c = wq_c.reshape(QC, D)
        wk_c = wk[c * HD:(c + 1) * HD][perm, :]
        wv_c = wv[c * HD:(c + 1) * HD]
        wkv_c = np.concatenate([wk_c, wv_c], axis=0)          # [128, D]
        wo_c = wo[:, c * QC:(c + 1) * QC]                      # [D, QC]
        woT3 = _to_bf16(
            np.ascontiguousarray(wo_c.T).reshape(2, 128, D)
            .transpose(1, 0, 2))                               # [128, 2, D]
        in_maps.append({
            "xTt": xTt,
            "wqT": _to_bf16(wq_c.T),
            "wkvT": _to_bf16(wkv_c.T),
            "woT3": woT3,
            "cosT": cosT,
            "sinT": sinT,
        })
    return in_maps


def _bf16_to_f32(a):
    return (a.view(np.uint16).astype(np.uint32) << 16).view(np.float32)


def _assemble(res):
    """Sum the 8 per-core tiled partials into the full [1, T, D] output."""
    acc = np.zeros((T // 128, 4, 128, S), dtype=np.float32)
    for c in range(NCORES):
        r = np.asarray(res.results[c]["out"])
        acc += _bf16_to_f32(r).reshape(T // 128, 4, 128, S)
    return acc.transpose(0, 2, 1, 3).reshape(1, T, D)


def kernel(x, wq, wk, wv, wo):
    from concourse.bass_utils import run_bass_kernel_spmd

    nc = _get_nc()
    in_maps = _prep_inputs(x, wq, wk, wv, wo)
    res = run_bass_kernel_spmd(nc, in_maps, core_ids=list(range(NCORES)))
    return _assemble(res)


# revision 35
# speedup vs baseline: 1.1384x; 1.1384x over previous
"""Causal self-attention (GQA + RoPE) for TRN2, sharded over 8 NeuronCores.

Sharding: tensor-parallel over heads. Each core owns 4 query heads and 1 KV
head (H=32, HKV=8 -> group size 4). Column-parallel q/k/v projections,
row-parallel o_proj; the final all-reduce over the 8 partial [T, D] outputs
happens on the host after the gather.

v2 (bf16 pipeline):
  - All matmul operands are bf16 (PSUM accumulation stays fp32). bf16 enables
    Fast Weight Load on LDWEIGHTS (fp32/f32r is excluded from FWL), halves
    SBUF/DMA traffic, and doubles DVE throughput on SBUF-only elementwise ops.
  - Attention runs per head-PAIR (heads 2hp, 2hp+1 live on partition halves
    0:64 / 64:128 of qT[hp]): the two score matmuls of a pair use disjoint
    PE row-groups (tile_position rows 0 vs 64) and overlap on the array.
  - exp runs once per (pair, key-block) over a [128, 2, 512] PSUM group,
    amortizing the ACT engine's 352-cycle pipeline-fill across both heads.
  - Scores are computed transposed (scoresT [s, t]) so the softmax denominator
    rides the attn@v matmul via a ones-column appended to the v stationary.
  - Causal masking: above-diagonal s-blocks are skipped; diagonal blocks get
    one affine_select over both heads post-exp.
  - 1/denom (from ACT's LUT reciprocal) is broadcast across partitions with
    gpsimd.partition_broadcast, keeping the PE and PSUM out of the epilogue.
  - o_proj evictions run on DVE (ACT is loaded with exp); out is written bf16
    in a tiled [ti, n, p, s] DRAM layout (contiguous 128KB stores) and
    assembled + reduced on the host.
  - x is staged in DRAM pre-tiled per (k-chunk, strip) so every activation
    load is one contiguous 128KB DMA.
"""

import math

import numpy as np

import concourse.bass as bass
import concourse.mybir as mybir
import concourse.tile as tile
from concourse import bacc
from concourse.masks import make_identity

D = 2048
H = 32
HKV = 8
HD = 64
T = 2048
NCORES = 8
HPC = H // NCORES        # 4 query heads per core
QC = HPC * HD            # 256 q dims per core
ROPE_BASE = 10000.0
S = 512                  # t-strip / moving-operand width
NSTRIP = T // S          # 4
KC = D // 128            # 16 contraction chunks

F32 = mybir.dt.float32
F32R = mybir.dt.float32r
BF16 = mybir.dt.bfloat16


def _build_kernel(debug=False):
    nc = bacc.Bacc("TRN2", target_bir_lowering=False, debug=False,
                   num_devices=NCORES)

    # x pre-tiled: rows (kc*NSTRIP + strip)*128 + p, cols s
    xTt = nc.dram_tensor("xTt", [KC * NSTRIP * 128, S], BF16,
                         kind="ExternalInput").ap()
    wqT = nc.dram_tensor("wqT", [D, QC], BF16, kind="ExternalInput").ap()
    wkvT = nc.dram_tensor("wkvT", [D, 128], BF16, kind="ExternalInput").ap()
    woT3 = nc.dram_tensor("woT3", [128, 2, D], BF16, kind="ExternalInput").ap()
    cosT = nc.dram_tensor("cosT", [128, T], BF16, kind="ExternalInput").ap()
    sinT = nc.dram_tensor("sinT", [128, T], BF16, kind="ExternalInput").ap()
    # out pre-tiled: rows ((ti*4 + n)*128 + p), cols s  (ti = global 128-row
    # tile of t, n = 512-col block of d)
    out = nc.dram_tensor("out", [T // 128 * 4 * 128, S], BF16,
                         kind="ExternalOutput").ap()
    dbg = {}
    if debug:
        for nm, shp, dt in [("d_qT0", [128, T], BF16), ("d_qT1", [128, T], BF16),
                            ("d_kT", [128, T], BF16),
                            ("d_vaug", [128, 16 * 65], BF16),
                            ("d_ex", [128, 2 * 4 * S], BF16),
                            ("d_yt", [128, 2 * S], F32),
                            ("d_dn", [128, S], F32), ("d_bc", [128, S], F32),
                            ("d_ytn0", [128, S], BF16),
                            ("d_ytn1", [128, S], BF16)]:
            dbg[nm] = nc.dram_tensor(nm, shp, dt, kind="ExternalOutput").ap()

    def act_reciprocal(out_ap, in_ap):
        """ACT-engine reciprocal via direct InstActivation emission.
        bass gatekeeps ActivationFunctionType.Reciprocal behind a precision
        advisory; for softmax denominators the LUT accuracy (~1e-5) is far
        below the bf16 noise floor, and ACT has idle slots between exps."""
        eng = nc.scalar
        ins = [eng.lower_ap(in_ap)]
        for v in (0.0, 1.0, 0.0):  # bias, scale, alpha
            ins.append(mybir.ImmediateValue(dtype=mybir.dt.float32, value=v))
        return eng.add_instruction(mybir.InstActivation(
            name=nc.get_next_instruction_name(),
            func=mybir.ActivationFunctionType.Reciprocal,
            ins=ins, outs=[eng.lower_ap(out_ap)]))

    with tile.TileContext(nc) as tc:
        with (
            tc.tile_pool(name="consts", bufs=1) as consts,
            tc.tile_pool(name="persist", bufs=1) as persist,
            tc.tile_pool(name="xa", bufs=50) as xap,
            tc.tile_pool(name="rtmp", bufs=6) as rtmp,
            tc.tile_pool(name="swp", bufs=4) as swp,
            tc.tile_pool(name="vtmp", bufs=2) as vtmp,
            tc.tile_pool(name="expp", bufs=4) as expp,
            tc.tile_pool(name="ytn", bufs=6) as ytnp,
            tc.tile_pool(name="outst", bufs=4) as outst,
            tc.tile_pool(name="dn", bufs=4) as dnp,
            tc.tile_pool(name="psA", bufs=1, space="PSUM") as psA,   # 4KB
            tc.tile_pool(name="psY", bufs=2, space="PSUM") as psY,   # 8KB
            tc.tile_pool(name="psP", bufs=1, space="PSUM") as psP,   # 2KB
            tc.tile_pool(name="psO", bufs=1, space="PSUM") as psO,   # 2KB
        ):
            # ---- constants; DMA issue order interleaved per k-chunk so the
            # first projection matmul only waits on chunk 0 of wq/x ----
            wq_sb = consts.tile([128, KC, QC], BF16)
            wkv_sb = consts.tile([128, KC, 128], BF16)
            xa_strips = {}

            def load_xa(strip):
                xa = []
                for kc in range(KC):
                    xt = xap.tile([128, S], BF16, tag="xa",
                                  name=f"xa{strip}_{kc}")
                    r0 = (kc * NSTRIP + strip) * 128
                    nc.sync.dma_start(out=xt, in_=xTt[r0:r0 + 128, :])
                    xa.append(xt)
                xa_strips[strip] = xa

            xa0 = []
            for kc in range(KC):
                nc.sync.dma_start(
                    out=wq_sb[:, kc, :], in_=wqT[kc * 128:(kc + 1) * 128, :])
                nc.sync.dma_start(
                    out=wkv_sb[:, kc, :], in_=wkvT[kc * 128:(kc + 1) * 128, :])
                xt = xap.tile([128, S], BF16, tag="xa", name=f"xa0_{kc}")
                r0 = (kc * NSTRIP + 0) * 128
                nc.sync.dma_start(out=xt, in_=xTt[r0:r0 + 128, :])
                xa0.append(xt)
            xa_strips[0] = xa0
            cs_c = consts.tile([128, T], BF16)
            cs_s = consts.tile([128, T], BF16)
            nc.sync.dma_start(out=cs_c, in_=cosT)
            nc.sync.dma_start(out=cs_s, in_=sinT)
            load_xa(1)
            wo_sb = consts.tile([128, 2, D], BF16)
            nc.gpsimd.dma_start(out=wo_sb, in_=woT3)
            ident = consts.tile([128, 128], F32)
            make_identity(nc, ident)
            # PE warmup: ~3.4us of array activity lifts the HAM 1.2GHz cold
            # throttle while the first input DMAs land (fp32 on purpose:
            # 4 cyc/row keeps the array busy longer per instruction)
            warm_ps = psO.tile([128, S], F32, tag="po", name="warm")
            junk = consts.tile([128, S], F32)
            nc.vector.memset(junk, 1.0)
            for w in range(7):
                nc.tensor.matmul(
                    warm_ps, ident, junk,
                    start=True, stop=True, skip_group_check=True)

            # persistent activations
            qT = [persist.tile([128, T], BF16, tag=f"qT{i}", name=f"qT{i}")
                  for i in range(2)]
            # k duplicated on both partition halves so each q head can use
            # a stationary slice whose base partition matches its rhs base
            kT = persist.tile([128, T], BF16)
            vaug = persist.tile([128, 4 * NSTRIP, 65], BF16)
            ones_col = consts.tile([128, 4 * NSTRIP, 1], BF16)
            nc.vector.memset(ones_col, 1.0)
            nc.vector.tensor_copy(vaug[:, :, 64:65], ones_col)
            ones_f = consts.tile([128, 64], F32)
            nc.vector.memset(ones_f, 1.0)
            ones_r = consts.tile([128, 64], F32R)
            nc.vector.tensor_copy(ones_r, ones_f)

            def proj_filler(strip):
                """Yield closures, each emitting one PE op of this strip's
                q/kv projection; rope/evict DVE work rides along after the
                last matmul of each accumulation group."""
                t0 = strip * S
                tsl = slice(t0, t0 + S)
                xa = xa_strips[strip]

                def rope_q(hp, pq):
                    qc = rtmp.tile([128, S], BF16, tag="rtmp",
                                   name=f"qc{strip}{hp}")
                    qs = rtmp.tile([128, S], BF16, tag="rtmp",
                                   name=f"qs{strip}{hp}")
                    nc.vector.tensor_mul(qc, pq, cs_c[:, tsl])
                    nc.vector.tensor_mul(qs, pq, cs_s[:, tsl])
                    sw = swp.tile([128, S], BF16, tag="swp",
                                  name=f"sw{strip}{hp}")
                    for b in range(2):
                        nc.gpsimd.dma_start(
                            out=sw[b * 64:b * 64 + 32, :],
                            in_=qs[b * 64 + 32:b * 64 + 64, :])
                        nc.gpsimd.dma_start(
                            out=sw[b * 64 + 32:b * 64 + 64, :],
                            in_=qs[b * 64:b * 64 + 32, :])
                    nc.vector.tensor_add(qT[hp][:, tsl], qc, sw)

                for hp in range(2):
                    pool = psP if hp == 0 else psO
                    tag = "proj" if hp == 0 else "po"
                    pq = pool.tile([128, S], F32, tag=tag,
                                   name=f"pq{strip}_{hp}")
                    for kc in range(KC):
                        def mk(hp=hp, pq=pq, kc=kc):
                            nc.tensor.matmul(
                                pq, wq_sb[:, kc, hp * 128:(hp + 1) * 128],
                                xa[kc], start=(kc == 0), stop=(kc == KC - 1))
                            if kc == KC - 1:
                                rope_q(hp, pq)
                        yield mk

                pkv = psP.tile([128, S], F32, tag="proj", name=f"pkv{strip}")

                def rope_kv():
                    kc_t = rtmp.tile([128, S], BF16, tag="rtmp",
                                     name=f"kc{strip}")
                    ks_t = rtmp.tile([128, S], BF16, tag="rtmp",
                                     name=f"ks{strip}")
                    nc.vector.tensor_mul(
                        kc_t[0:64, :], pkv[0:64, :], cs_c[0:64, tsl])
                    nc.vector.tensor_mul(
                        ks_t[0:64, :], pkv[0:64, :], cs_s[0:64, tsl])
                    swk = swp.tile([128, S], BF16, tag="swp",
                                   name=f"swk{strip}")
                    nc.gpsimd.dma_start(out=swk[0:32, :], in_=ks_t[32:64, :])
                    nc.gpsimd.dma_start(out=swk[32:64, :], in_=ks_t[0:32, :])
                    nc.vector.tensor_add(
                        kT[0:64, tsl], kc_t[0:64, :], swk[0:64, :])
                    nc.gpsimd.dma_start(out=kT[64:128, tsl], in_=kT[0:64, tsl])
                    vt_s = vtmp.tile([128, S], F32, tag="vtmp",
                                     name=f"vt{strip}")
                    nc.vector.tensor_copy(vt_s[64:128, :], pkv[64:128, :])
                    return vt_s

                state = {}
                for kc in range(KC):
                    def mk(kc=kc):
                        nc.tensor.matmul(
                            pkv, wkv_sb[:, kc, :], xa_strips[strip][kc],
                            start=(kc == 0), stop=(kc == KC - 1))
                        if kc == KC - 1:
                            state["vt_s"] = rope_kv()
                    yield mk
                for n in range(4):
                    def mk(n=n):
                        pt = psO.tile([128, S], F32, tag="po",
                                      name=f"pt{strip}{n}")
                        nc.tensor.transpose(
                            pt[:, 0:64],
                            state["vt_s"][64:128, n * 128:(n + 1) * 128],
                            ident[64:128, 64:128])
                        nc.vector.tensor_copy(
                            vaug[:, strip * 4 + n, 0:64], pt[:, 0:64])
                    yield mk

            def oproj_filler(strip, ytn, alt=False):
                """Yield closures, each emitting one o_proj matmul; the
                eviction + store ride along after each group's stop. With
                alt=True (legal only while no projection holds psP) po tiles
                alternate between the two 1-buf pools so consecutive tiles
                don't serialize on each other's eviction."""
                for tsub in range(4):
                    ti = strip * 4 + tsub
                    for n in range(4):
                        if alt and n % 2 == 1:
                            po = psP.tile([128, S], F32, tag="proj",
                                          name=f"po{strip}{tsub}{n}")
                        else:
                            po = psO.tile([128, S], F32, tag="po",
                                          name=f"po{strip}{tsub}{n}")
                        for c in range(2):
                            def mk(po=po, c=c, tsub=tsub, n=n, ti=ti):
                                nc.tensor.matmul(
                                    po,
                                    ytn[c][:, tsub * 128:(tsub + 1) * 128],
                                    wo_sb[:, c, n * S:(n + 1) * S],
                                    start=(c == 0), stop=(c == 1),
                                    skip_group_check=True)
                                if c == 1:
                                    ot = outst.tile(
                                        [128, S], BF16, tag="out",
                                        name=f"ot{strip}{tsub}{n}")
                                    nc.vector.tensor_copy(ot, po)
                                    r0 = (ti * 4 + n) * 128
                                    nc.sync.dma_start(
                                        out=out[r0:r0 + 128, :], in_=ot)
                            yield mk

            def run_filler(filler, frac):
                import itertools
                for fn in itertools.islice(filler, frac):
                    fn()

            # strip 0 projection runs dense (nothing to overlap with)
            for fn in proj_filler(0):
                fn()

            ytn_strips = {}
            fillers = []

            for strip in range(NSTRIP):
                t0 = strip * S
                n_sc = (strip + 1) * 4
                ytn = [ytnp.tile([128, S], BF16, tag="ytn",
                                 name=f"ytn{strip}{i}") for i in range(2)]
                ytn_strips[strip] = ytn

                # filler schedule, balanced so the long strip-3 chain still
                # has PE work to hide exp latency: strip0 <- proj(1);
                # strip1 <- proj(2)+oproj(0); strip2 <- proj(3);
                # strip3 <- oproj(1)+oproj(2); tail <- oproj(3)
                pending = 0
                if strip + 1 < NSTRIP:
                    if strip + 2 < NSTRIP:
                        load_xa(strip + 2)
                    fillers.append(proj_filler(strip + 1))
                    pending += 52
                for op_strip in ([0] if strip == 1 else
                                 [1, 2] if strip == 3 else []):
                    fillers.append(oproj_filler(op_strip,
                                                ytn_strips[op_strip],
                                                alt=(strip == 3)))
                    pending += 32

                n_chunks = 2 * n_sc

                import itertools
                filler_iter = itertools.chain(*fillers)
                fillers = [filler_iter]
                emitted = [0]
                chunk_i = [0]

                def chunk_filler():
                    # front-loaded drain: fillers done by ~85% of chunks so
                    # rope/o_proj chains complete well before the strip ends
                    chunk_i[0] += 1
                    tgt = min(n_chunks, chunk_i[0] * 5 // 4)
                    want = pending * tgt // n_chunks - emitted[0]
                    emitted[0] += want
                    run_filler(filler_iter, want)

                ytps = []
                for hp in range(2):          # head pair (2hp, 2hp+1)
                    ytp = psY.tile([128, 2, S], F32, tag="yt",
                                   name=f"ytp{strip}{hp}")
                    ytps.append(ytp)
                    for j in range(n_sc):
                        o = max(j * 128 - t0, 0)
                        jb = slice(j * 128, (j + 1) * 128)
                        # in the projection-free last strip, odd key-blocks
                        # borrow the two idle 1-bank pools so consecutive
                        # blocks' scores don't serialize on a single psA
                        # buffer behind exp
                        split = (strip == NSTRIP - 1) and (j % 2 == 1)
                        if split:
                            pss = [psO.tile([128, S], F32, tag="po",
                                            name=f"sA{strip}{hp}{j}"),
                                   psP.tile([128, S], F32, tag="proj",
                                            name=f"sB{strip}{hp}{j}")]
                            ps_h = [pss[0][:, o:S], pss[1][:, o:S]]
                        else:
                            ps = psA.tile([128, 2, S], F32, tag="ps",
                                          name=f"s{strip}{hp}{j}")
                            ps_h = [ps[:, 0, o:S], ps[:, 1, o:S]]
                        # the two heads use disjoint PE row groups (0 / 64)
                        # and overlap on the array
                        nc.tensor.matmul(
                            ps_h[0], kT[0:64, jb],
                            qT[hp][0:64, t0 + o:t0 + S],
                            start=True, stop=True)
                        nc.tensor.matmul(
                            ps_h[1], kT[64:128, jb],
                            qT[hp][64:128, t0 + o:t0 + S],
                            start=True, stop=True)
                        ex = expp.tile([128, 2, S], BF16, tag="exp",
                                       name=f"e{strip}{hp}{j}")
                        if split:
                            for h01 in range(2):
                                nc.scalar.activation(
                                    ex[:, h01, o:S], ps_h[h01],
                                    mybir.ActivationFunctionType.Exp,
                                    scale=1.0 / math.sqrt(HD))
                        else:
                            nc.scalar.activation(
                                ex[:, :, o:S], ps[:, :, o:S],
                                mybir.ActivationFunctionType.Exp,
                                scale=1.0 / math.sqrt(HD))
                        if j * 128 - t0 >= 0:
                            nc.gpsimd.affine_select(
                                out=ex[:, :, o:o + 128],
                                in_=ex[:, :, o:o + 128],
                                pattern=[[0, 2], [1, 128]], base=0,
                                channel_multiplier=-1,
                                compare_op=mybir.AluOpType.is_ge, fill=0.0)
                        if debug and strip == 0 and hp == 0:
                            nc.sync.dma_start(
                                out=dbg["d_ex"].rearrange(
                                    "p (j h s) -> p j h s", j=4, h=2)[:, j],
                                in_=ex)
                        chunk_filler()
                        nc.tensor.matmul(
                            ytp[0:65, 0, o:S], vaug[:, j, :], ex[:, 0, o:S],
                            start=(j == 0), stop=(j == n_sc - 1),
                            skip_group_check=True)
                        nc.tensor.matmul(
                            ytp[0:65, 1, o:S], vaug[:, j, :], ex[:, 1, o:S],
                            start=(j == 0), stop=(j == n_sc - 1),
                            skip_group_check=True)

                if debug and strip == 0:
                    ydump = dnp.tile([128, 2, S], F32, tag="ydump",
                                     bufs=1, name="ydump")
                    nc.vector.tensor_copy(ydump, ytps[0])
                    nc.sync.dma_start(
                        out=dbg["d_yt"],
                        in_=ydump.rearrange("p a b -> p (a b)"))
                # strip epilogue: 1/denom for all 4 heads via ACT LUT recip
                # (two adjacent instructions -> one Exp->Recip->Exp table
                # round-trip per strip), broadcast across partitions with PE
                # outer products (ones[64] x recip-row) into the scores pool
                # banks, then normalize on DVE
                dds = []
                for hp in range(2):
                    dd = dnp.tile([128, 2, S], F32R, tag="dd", bufs=2,
                                  name=f"dd{strip}{hp}")
                    nc.scalar.copy(dd[64:65, :, :], ytps[hp][64:65, :, :])
                    dds.append(dd)
                run_filler(filler_iter, 5)
                for hp in range(2):
                    bc_ps = psA.tile([128, 2, S], F32, tag="ps",
                                     name=f"bcp{strip}{hp}")
                    for h01 in range(2):
                        nc.tensor.matmul(
                            bc_ps[0:64, h01, :], ones_r[64:65, :],
                            dds[hp][64:65, h01, :],
                            start=True, stop=True, skip_group_check=True)
                    bc_t = dnp.tile([128, 2, S], F32, tag="dn", bufs=2,
                                    name=f"bc{strip}{hp}")
                    nc.scalar.copy(bc_t[0:64, :, :], bc_ps[0:64, :, :])
                    # the custom-DVE approx recip needs a base-0 AP (it
                    # mis-executes on partition-offset slices), so invert the
                    # broadcast denominator on 64 lanes
                    bc_r = dnp.tile([128, 2, S], F32, tag="dr", bufs=2,
                                    name=f"br{strip}{hp}")
                    nc.vector.reciprocal_approx_fast(
                        out=bc_r[0:64, :, :], in_=bc_t[0:64, :, :])
                    nc.vector.tensor_mul(
                        ytn[hp][0:64, :], ytps[hp][0:64, 0, :],
                        bc_r[0:64, 0, :])
                    ntmp = dnp.tile([128, S], BF16, tag="ntmp",
                                    bufs=2, name=f"nt{strip}{hp}")
                    nc.vector.tensor_mul(
                        ntmp[0:64, :], ytps[hp][0:64, 1, :], bc_r[0:64, 1, :])
                    nc.gpsimd.dma_start(
                        out=ytn[hp][64:128, :], in_=ntmp[0:64, :])
                    if debug and strip == 0 and hp == 0:
                        nc.sync.dma_start(out=dbg["d_dn"],
                                          in_=bc_r[:, 0, :])
                        nc.sync.dma_start(out=dbg["d_bc"], in_=bc_t[:, 0, :])

                if debug and strip == 0:
                    nc.sync.dma_start(out=dbg["d_qT0"], in_=qT[0])
                    nc.sync.dma_start(out=dbg["d_qT1"], in_=qT[1])
                    nc.sync.dma_start(out=dbg["d_kT"], in_=kT)
                    nc.sync.dma_start(
                        out=dbg["d_vaug"],
                        in_=vaug.rearrange("p a b -> p (a b)"))
                    nc.sync.dma_start(out=dbg["d_ytn0"], in_=ytn[0])
                    nc.sync.dma_start(out=dbg["d_ytn1"], in_=ytn[1])

                # drain any leftover filler before the next strip
                for fn in filler_iter:
                    fn()
                fillers = []

            # last strip's o_proj runs dense at the tail
            for fn in oproj_filler(NSTRIP - 1, ytn_strips[NSTRIP - 1],
                                   alt=True):
                fn()

    nc.compile()
    return nc


_NC_CACHE = None


def _get_nc():
    global _NC_CACHE
    if _NC_CACHE is None:
        _NC_CACHE = _build_kernel()
    return _NC_CACHE


def _to_bf16(a):
    import ml_dtypes
    return np.ascontiguousarray(a).astype(ml_dtypes.bfloat16)


def _prep_inputs(x, wq, wk, wv, wo):
    """Host-side shard + layout prep. Returns per-core input maps."""
    x = np.asarray(x, dtype=np.float32).reshape(T, D)
    wq = np.asarray(wq, dtype=np.float32)
    wk = np.asarray(wk, dtype=np.float32)
    wv = np.asarray(wv, dtype=np.float32)
    wo = np.asarray(wo, dtype=np.float32)

    xT = np.ascontiguousarray(x.T)  # [D, T]
    xTt = _to_bf16(
        xT.reshape(KC, 128, NSTRIP, S).transpose(0, 2, 1, 3)
        .reshape(KC * NSTRIP * 128, S))

    # head-dim permutation for rope: [even pair comps | odd pair comps]
    perm = np.concatenate([np.arange(0, HD, 2), np.arange(1, HD, 2)])

    # rope tables in the [d, t] layout
    theta = 1.0 / ROPE_BASE ** (np.arange(0, HD, 2, dtype=np.float64) / HD)
    ang = np.arange(T, dtype=np.float64)[None, :] * theta[:, None]  # [32, T]
    cos_blk = np.cos(ang).astype(np.float32)
    sin_blk = np.sin(ang).astype(np.float32)
    cosT = _to_bf16(np.tile(np.concatenate([cos_blk, cos_blk], 0), (2, 1)))
    sinT = _to_bf16(np.tile(np.concatenate([sin_blk, -sin_blk], 0), (2, 1)))

    in_maps = []
    for c in range(NCORES):
        wq_c = wq[c * QC:(c + 1) * QC].reshape(HPC, HD, D)[:, perm, :]
        wq_c = wq_c.reshape(QC, D)
        wk_c = wk[c * HD:(c + 1) * HD][perm, :]
        wv_c = wv[c * HD:(c + 1) * HD]
        wkv_c = np.concatenate([wk_c, wv_c], axis=0)          # [128, D]
        wo_c = wo[:, c * QC:(c + 1) * QC]                      # [D, QC]
        woT3 = _to_bf16(
            np.ascontiguousarray(wo_c.T).reshape(2, 128, D)
            .transpose(1, 0, 2))                               # [128, 2, D]
        in_maps.append({
            "xTt": xTt,
            "wqT": _to_bf16(wq_c.T),
            "wkvT": _to_bf16(wkv_c.T),
            "woT3": woT3,
            "cosT": cosT,
            "sinT": sinT,
        })
    return in_maps


def _bf16_to_f32(a):
    return (a.view(np.uint16).astype(np.uint32) << 16).view(np.float32)


def _assemble(res):
    """Sum the 8 per-core tiled partials into the full [1, T, D] output."""
    acc = np.zeros((T // 128, 4, 128, S), dtype=np.float32)
    for c in range(NCORES):
        r = np.asarray(res.results[c]["out"])
        acc += _bf16_to_f32(r).reshape(T // 128, 4, 128, S)
    return acc.transpose(0, 2, 1, 3).reshape(1, T, D)


def kernel(x, wq, wk, wv, wo):
    from concourse.bass_utils import run_bass_kernel_spmd

    nc = _get_nc()
    in_maps = _prep_inputs(x, wq, wk, wv, wo)
    res = run_bass_kernel_spmd(nc, in_maps, core_ids=list(range(NCORES)))
    return _assemble(res)
